# revision 1
# baseline (speedup 1.0000x reference)
"""Trainium2 Bass kernel for nn_CenterAwarePseudoModule (retrieval_knn).

Reference (per row i of feats, per centroid j):
    f_i   = [feats_i, 1] / ||[feats_i, 1]||
    d2_ij = ||f_i||^2 + ||c_j||^2 - 2 f_i . c_j
    out_i = labelset[argmin_j sqrt(max(d2_ij, 0))]

With q_i = ||feats_i||^2 + 1, h_j = ||c_j||^2 (full row incl. bias col),
G_ij = feats_i . c_j[:D], cb_j = c_j[D]:
    argmin_j d2 = argmax_j (G_ij + cb_j - rh_i * h_j),   rh_i = sqrt(q_i)/2
(positive per-row affine transforms preserve the argmin; validated
empirically against the fp64 oracle: 0 mismatches).

Device strategy (data-parallel over 8 NeuronCores, rows sharded):
  - G via fp8(e4m3) matmuls in DoubleRow perf mode: contraction 256/inst
    at 0.5 cycles/row (2x bf16 PE rate), two 512-col moving chunks per
    group (the ISA 512-moving-element cap; walrus's LDW dedup rejects
    perf-mode LDWEIGHTS, so each matmul self-loads its stationary).
  - bias (cb - rh*h) folded into PSUM by a tiny fp32r matmul first:
    stationary [3,128] = [ones; rh; rh], moving [3,1024] = [cb; -h_hi; -h_lo]
    (h split so fp32r's reduced mantissa on h stays exact).
  - epilogue per 128-row tile: vector.max + max_index straight off PSUM
    [128,1024] (cols >=1000 padded to lose by construction), DMA the TOP-2
    indices out.
  - prologue: bias matmuls for m0-m3 run off the tiny rh/bias DMAs while
    ct/ft stream in; k-major order over m0-m2 tracks the ct prefetch;
    everything is SBUF-resident afterwards (fp8 inputs: 6.3MB/core total).
Host does layout prep (transpose/tiling, e4m3 rounding, norms), an exact
fp64 re-score of each row's device top-2 (so fp8 matmul noise cannot flip
the argmin: a true winner outside the device top-2 needs two independent
>4-sigma fp8 noise events), and the final labelset gather.
"""
import sys

sys.path.insert(0, "/opt/trn_rl_repo")

import numpy as np
import ml_dtypes

N, D, NCENT = 16384, 2048, 1000
NC1024 = 1024            # centroid dim padded to 8 psum chunks of 256
NCORES = 8
R = N // NCORES          # rows per core = 2048
MT = R // 128            # m-tiles per core = 16
KG = D // 256            # DoubleRow contraction groups = 8
HPAD = -2500.0           # pad "-h" value: loses by ~rh*650 for every row

_cache = {}


def _build():
    import concourse.bacc as bacc
    import concourse.tile as tile
    from concourse import mybir

    dt = mybir.dt
    DR = mybir.MatmulPerfMode.DoubleRow

    nc = bacc.Bacc("TRN2", target_bir_lowering=False, debug=False)

    ft = nc.dram_tensor("ft", [MT, 128, KG, 2, 128], dt.float8e4, kind="ExternalInput")
    ct = nc.dram_tensor("ct", [128, KG, 2, NC1024], dt.float8e4, kind="ExternalInput")
    bmv = nc.dram_tensor("bmv", [3, NC1024], dt.float32r, kind="ExternalInput")
    rhd = nc.dram_tensor("rh", [3, MT * 128], dt.float32r, kind="ExternalInput")
    outp = nc.dram_tensor("pred", [MT, 128, 2], dt.uint32, kind="ExternalOutput")

    with tile.TileContext(nc) as tc:
        with (
            tc.tile_pool(name="const", bufs=1) as constp,
            tc.tile_pool(name="epi", bufs=3) as epi,
            tc.tile_pool(name="ps", bufs=4, space="PSUM") as psp,
        ):
            # ---- prologue DMA: tiny bias/rh first (unblocks the PE at t~0),
            # then ct g0 + the first two ft tiles, then the rest of ct, then
            # the rest of ft. Everything is resident for the whole kernel. ----
            bias_sb = constp.tile([3, NC1024], dt.float32r, tag="bias")
            nc.sync.dma_start(bias_sb[:], bmv.ap())
            rh_sb = constp.tile([3, MT * 128], dt.float32r, tag="rh")
            nc.sync.dma_start(rh_sb[:], rhd.ap())
            ct_tiles = [
                constp.tile([128, 2, NC1024], dt.float8e4, tag=f"ct{g}",
                            name=f"ctt{g}")
                for g in range(KG)
            ]
            ft_tiles = [
                constp.tile([128, KG, 2, 128], dt.float8e4, tag=f"ft{m}",
                            name=f"ftt{m}")
                for m in range(MT)
            ]
            # ct streamed in 512-col halves: the ch0 matmul of a group only
            # depends on the first half, so the k-major warm-up starts each
            # group ~0.4us earlier than with whole-tile arrivals.
            def ct_dma(g):
                for ch in range(2):
                    nc.sync.dma_start(
                        ct_tiles[g][:, :, ch * 512:(ch + 1) * 512],
                        ct.ap()[:, g, :, ch * 512:(ch + 1) * 512],
                    )

            ct_dma(0)
            for m in range(3):
                nc.sync.dma_start(ft_tiles[m][:], ft.ap()[m])
            for g in range(1, KG):
                ct_dma(g)
            for m in range(3, MT):
                nc.sync.dma_start(ft_tiles[m][:], ft.ap()[m])

            def bias_mm(ps, m):
                lhs = rh_sb[:, m * 128:(m + 1) * 128]
                nc.tensor.matmul(
                    ps[:, 0:512], lhs, bias_sb[:, 0:512],
                    start=True, stop=False,
                )
                nc.tensor.matmul(
                    ps[:, 512:NC1024], lhs, bias_sb[:, 512:NC1024],
                    start=True, stop=False,
                )

            def g_group(ps, m, g):
                lhs = ft_tiles[m][:, g]
                for ch in range(2):
                    nc.tensor.matmul(
                        ps[:, ch * 512:(ch + 1) * 512],
                        lhs,
                        ct_tiles[g][:, :, ch * 512:(ch + 1) * 512],
                        start=False, stop=(g == KG - 1),
                        perf_mode=DR,
                    )

            def epilogue(ps, m):
                mx = epi.tile([128, 8], dt.float32, tag="mx", name=f"mx{m}")
                nc.vector.max(mx[:], ps[:])
                mi = epi.tile([128, 8], dt.uint32, tag="mi", name=f"mi{m}")
                nc.vector.max_index(mi[:], mx[:], ps[:])
                nc.sync.dma_start(outp.ap()[m], mi[:, 0:2])

            # ---- warm-up: bias matmuls for m0-m3 need only the tiny rh/bias
            # DMAs, so they fill the PE while ct/ft stream in; then m0-m2
            # k-major tracks the ct prefetch. ----
            pss = [
                psp.tile([128, NC1024], dt.float32, tag="ps", name=f"ps{m}")
                for m in range(4)
            ]
            for m in range(4):
                bias_mm(pss[m], m)
            for g in range(KG):
                for m in range(3):
                    g_group(pss[m], m, g)
            for m in range(3):
                epilogue(pss[m], m)

            # ---- steady state: m-major (m3's psum is already bias-primed) --
            for m in range(3, MT):
                if m == 3:
                    ps = pss[3]
                else:
                    ps = psp.tile([128, NC1024], dt.float32, tag="ps",
                                  name=f"ps{m}")
                    bias_mm(ps, m)
                for g in range(KG):
                    g_group(ps, m, g)
                epilogue(ps, m)

    nc.compile()
    return nc


def _prep_inputs(feats, initc):
    feats = np.ascontiguousarray(np.asarray(feats, dtype=np.float32))
    initc = np.ascontiguousarray(np.asarray(initc, dtype=np.float32))

    f8 = feats.astype(ml_dtypes.float8_e4m3)
    c8 = initc[:, :D].astype(ml_dtypes.float8_e4m3)

    # ct[p, g, i, j] = c8[j, g*256 + i*128 + p], zero-padded to 1024 centroids
    ctp = np.zeros((128, KG, 2, NC1024), dtype=ml_dtypes.float8_e4m3)
    ctp[:, :, :, :NCENT] = c8.T.reshape(KG, 2, 128, NCENT).transpose(2, 0, 1, 3)

    h = (initc.astype(np.float64) ** 2).sum(axis=1)
    # split h so the PE's reduced-mantissa fp32r input rounding is exact:
    # h_hi has 10 mantissa bits (exact under any >=10-bit PE rounding),
    # h_lo carries the remainder (|h_lo| ~ h * 2^-11, its own rounding moot)
    mant, expo = np.frexp(h)
    h_hi = np.ldexp(np.round(mant * 1024.0) / 1024.0, expo)
    h_lo = (h - h_hi).astype(np.float32)
    bmv = np.zeros((3, NC1024), dtype=np.float32)
    bmv[0, :NCENT] = initc[:, D]
    bmv[1, :NCENT] = -h_hi.astype(np.float32)
    bmv[1, NCENT:] = HPAD
    bmv[2, :NCENT] = -h_lo

    q = (feats.astype(np.float64) ** 2).sum(axis=1) + 1.0
    rh_all = (np.sqrt(q) / 2.0).astype(np.float32)  # [N]

    in_maps = []
    for c in range(NCORES):
        fc = f8[c * R:(c + 1) * R]  # [R, D]
        # ft[m, p, g, i, r] = fc[m*128 + r, (g*2+i)*128 + p]
        X = np.ascontiguousarray(
            fc.reshape(MT, 128, KG, 2, 128).transpose(0, 4, 2, 3, 1)
        )
        rhc = np.empty((3, MT * 128), dtype=np.float32)
        rhc[0] = 1.0
        rhc[1] = rh_all[c * R:(c + 1) * R]
        rhc[2] = rhc[1]
        in_maps.append({"ft": X, "ct": ctp, "bmv": bmv, "rh": rhc})
    return in_maps


def _enable_ldw_opt():
    """walrus dedups back-to-back LDWEIGHTS of the same stationary operand
    when --enable-ldw-opt=true; concourse hardcodes false. NOTE: walrus
    rejects DoubleRow InstLdweights under this flag ("not compatible with
    LDW optimization"), so the fp8 DoubleRow kernel must run without it."""
    import concourse.bass_utils as bu

    if getattr(bu, "_ldw_opt_patched", False):
        return
    orig = bu.run_command

    def patched(argv, **kw):
        argv = [
            "--enable-ldw-opt=true" if a == "--enable-ldw-opt=false" else a
            for a in argv
        ]
        return orig(argv, **kw)

    bu.run_command = patched
    bu._ldw_opt_patched = True


def _refine_top2(feats, initc, cand):
    """Exact (fp64) score comparison of the device's top-2 candidates per
    row; fixes any argmax flip the fp8 matmul noise may have caused. The
    true winner is in the device top-2 with overwhelming probability (a
    displacement needs two independent >4-sigma noise events)."""
    feats = np.asarray(feats, np.float64)
    initc = np.asarray(initc, np.float64)
    h = (initc * initc).sum(axis=1)
    cb = initc[:, D]
    rh = np.sqrt((feats * feats).sum(axis=1) + 1.0) / 2.0
    pred = np.empty(feats.shape[0], dtype=np.int64)
    CH = 2048
    for a in range(0, feats.shape[0], CH):
        b = a + CH
        c2 = initc[cand[a:b], :D]                      # [CH, 2, D]
        g = np.matmul(c2, feats[a:b, :, None])[..., 0]  # [CH, 2]
        s = g + cb[cand[a:b]] - rh[a:b, None] * h[cand[a:b]]
        pick = s[:, 1] > s[:, 0]
        pred[a:b] = np.where(pick, cand[a:b, 1], cand[a:b, 0])
    return pred


def _run(feats, initc, labelset, trace=False):
    from concourse.bass_utils import run_bass_kernel_spmd

    if "nc" not in _cache:
        _cache["nc"] = _build()
    nc = _cache["nc"]

    in_maps = _prep_inputs(feats, initc)
    res = run_bass_kernel_spmd(
        nc, in_maps, core_ids=list(range(NCORES)), trace=trace
    )

    cand = np.concatenate(
        [res.results[c]["pred"].reshape(R, 2) for c in range(NCORES)]
    ).astype(np.int64)
    preds = _refine_top2(feats, initc, cand)
    labelset = np.asarray(labelset)
    out = labelset[preds]
    return out, res


def kernel(feats, initc, labelset):
    out, _ = _run(feats, initc, labelset, trace=False)
    return out



# revision 13
# speedup vs baseline: 1.0692x; 1.0692x over previous
"""Trainium2 Bass kernel for nn_CenterAwarePseudoModule (retrieval_knn).

Reference (per row i of feats, per centroid j = initc[labelset]):
    f_i   = [feats_i, 1] / ||[feats_i, 1]||
    d2_ij = ||f_i||^2 + ||c_j||^2 - 2 f_i . c_j
    out_i = labelset[argmin_j sqrt(max(d2_ij, 0))]

Math (host-normalized rows -> constant bias row):
  With r_i = ||[feats_i,1]||, c = mean(r), ft'_i = feats_i * (c/r_i):
    argmin_j d2 = argmax_j [ (c/r_i)(G_ij + cb_j) - (c/2) h_j ]
  where G_ij = feats_i . cD_j, cb_j = c_j[D], h_j = ||c_j||^2.
  Approximating (c/r_i) cb_j ~= cb_j (error <= 0.2 vs fp8 matmul noise
  sigma ~2.7 and top-gap ~4.5) makes the non-matmul part a CONSTANT row:
    score_ij = ft'_i . cD_j + bias_j,   bias_j = cb_j - (c/2) h_j
  Device ships per-row top-8 indices per PSUM bank (cols 0:512, 512:1000);
  host re-scores the top per-bank candidates exactly in fp64 (validated:
  true winner is device rank 0 in 16383/16384 rows, rank 1 in the other).

Device kernel (8 cores, rows data-parallel; 2048 rows = 16 m-tiles/core):
  - PE does ONLY fp8(e4m3) DoubleRow matmuls (hw peak 157 TF/s: 216 ns per
    512-col chunk): 8 contraction groups x (512 + 488)-col chunks.
  - Bias stays off the PE entirely: PSUM holds G only (standard
    start=True..stop=True groups); DVE adds the constant bias row in the
    epilogue (tensor_add into SBUF scratch, then MAX8 on SBUF). Costs the
    same DVE time as scanning PSUM directly, and avoids the fp32r bias
    matmuls that burned ~5us of PE in the previous kernel.
  - Each PSUM bank is its own tile (tags psA/psB, ring 4 each): the bank-0
    epilogue (DVE) then overlaps the bank-1 k-loop (PE) without Tile's
    tile-granular WAR serialization (measured 0.9-2.7us/tile otherwise).
  - PE p-state warmup: dep-free dummy DR matmuls run during the launch
    dead time (~6.9-9us) so real matmuls start at the full 2.4 GHz clock.
    An Act dummy likewise pulls the lazy ACT_TABLE_LOAD off the path.
  - m0-m2 run k-major (3 matmuls per arriving ct group) to track the DMA
    stream; m3+ run m-major ch-blocked. One tile / one DMA writer each
    (Tile deps are unreliable with multiple DMA writers into one tile).
  - Epilogue per bank: MAX8 + MAX_INDEX -> staged in SBUF; ONE output DMA
    at the end (avoids 2048 8-byte descriptors dribbling into the final
    barrier).
Host does layout prep (transpose/tiling, e4m3 rounding, norms), the exact
fp64 re-score of each row's per-bank top candidates, and the final
labelset gather.
"""
import sys

sys.path.insert(0, "/opt/trn_rl_repo")

import numpy as np
import ml_dtypes

N, D, NCENT = 16384, 2048, 1000
NC0, NC1 = 512, 488      # psum bank split of the 1000 centroids
NCORES = 8
R = N // NCORES          # rows per core = 2048
MT = R // 128            # m-tiles per core = 16
KG = D // 256            # DoubleRow contraction groups = 8
NLB = 3                  # late-bias tiles (m0..m2): bias added by DVE
NWARM = 16               # p-state warmup matmuls (~110-400ns each)

_cache = {}


def _build():
    import concourse.bacc as bacc
    import concourse.tile as tile
    from concourse import mybir

    dt = mybir.dt
    DR = mybir.MatmulPerfMode.DoubleRow

    nc = bacc.Bacc("TRN2", target_bir_lowering=False, debug=False)

    ftd = nc.dram_tensor("ft", [MT, 128, KG, 2, 128], dt.float8e4,
                         kind="ExternalInput")
    ct0d = nc.dram_tensor("ct0", [KG, 128, 2, NC0], dt.float8e4,
                          kind="ExternalInput")
    ct1d = nc.dram_tensor("ct1", [KG, 128, 2, NC1], dt.float8e4,
                          kind="ExternalInput")
    brd = nc.dram_tensor("br", [128, NCENT], dt.float32, kind="ExternalInput")
    outp = nc.dram_tensor("pred", [128, MT * 2 * 8], dt.uint32,
                          kind="ExternalOutput")

    with tile.TileContext(nc) as tc:
        with (
            tc.tile_pool(name="const", bufs=1) as constp,
            tc.tile_pool(name="epi", bufs=3) as epi,
            tc.tile_pool(name="ps", bufs=4, space="PSUM") as psp,
        ):
            wa = constp.tile([128, 2, 128], dt.float8e4, tag="wa")
            ct0 = [constp.tile([128, 2, NC0], dt.float8e4, tag=f"ct0_{g}",
                               name=f"ct0t{g}")
                   for g in range(KG)]
            ct1 = [constp.tile([128, 2, NC1], dt.float8e4, tag=f"ct1_{g}",
                               name=f"ct1t{g}")
                   for g in range(KG)]
            ft = [constp.tile([128, KG, 2, 128], dt.float8e4, tag=f"ft{m}",
                              name=f"ftt{m}")
                  for m in range(MT)]
            br = constp.tile([128, NCENT], dt.float32, tag="br")
            stage = constp.tile([128, MT * 2 * 8], dt.uint32, tag="stage")
            scw = constp.tile([128, 8], dt.float32, tag="scw")

            # p-state warmups: memset a tiny tile on DVE, then dep-free DR
            # matmuls keep the PE busy from ~6.9us (barrier release) so the
            # DVFS ramp finishes before real data arrives. The Act dummy
            # pulls its lazy ACT_TABLE_LOAD (~1.1us) off the critical path.
            nc.vector.memset(wa[:], 0)
            # psum tiles are one full 2KB bank each (bank-aligned); bank B
            # uses only cols 0:NC1 of its 512-col tile.
            wpa = psp.tile([128, NC0], dt.float32, tag="psA", name="warmA")
            pa = [psp.tile([128, NC0], dt.float32, tag="psA", name=f"pa{m}")
                  for m in range(NLB)]
            pb = [psp.tile([128, NC0], dt.float32, tag="psB", name=f"pb{m}")
                  for m in range(NLB)]
            for w in range(NWARM):
                nc.tensor.matmul(
                    wpa[:, 0:128], wa[:], wa[:],
                    start=True, stop=True, perf_mode=DR,
                )
            nc.scalar.copy(scw[:], wa[:, 0, 0:8])

            # ---- DMA triggers (SP queue, ~0.6us each, issue order = need
            # order): ct0-g0 + ft-m0 first so the first real matmul fires
            # ~9us; ct0 groups interleave with ft m1-m3; br before the ct1
            # tail (first epilogues need it ~15.5us).
            def dma(dst, src):
                nc.sync.dma_start(dst, src)

            dma(ct0[0][:], ct0d.ap()[0])
            dma(ft[0][:], ftd.ap()[0])
            dma(ct0[1][:], ct0d.ap()[1])
            dma(ft[1][:], ftd.ap()[1])
            dma(ct0[2][:], ct0d.ap()[2])
            dma(ft[2][:], ftd.ap()[2])
            dma(ct0[3][:], ct0d.ap()[3])
            dma(ft[3][:], ftd.ap()[3])
            for g in range(4, KG):
                dma(ct0[g][:], ct0d.ap()[g])
            dma(br[:], brd.ap())
            for g in range(KG):
                dma(ct1[g][:], ct1d.ap()[g])
            for m in range(4, MT):
                dma(ft[m][:], ftd.ap()[m])

            def mm(ps, m, g, ch, start, stop, skip=False):
                rhs = ct0[g] if ch == 0 else ct1[g]
                out = ps[:] if ch == 0 else ps[:, 0:NC1]
                nc.tensor.matmul(
                    out, ft[m][:, g], rhs[:],
                    start=start, stop=stop, perf_mode=DR,
                    skip_group_check=skip,
                )

            def sview(m, b):
                o = (m * 2 + b) * 8
                return stage[:, o:o + 8]

            def epi_bank(ps, m, b, late_bias):
                lo, hi = (0, NC0) if b == 0 else (NC0, NCENT)
                w = hi - lo
                pv = ps[:] if b == 0 else ps[:, 0:NC1]
                if late_bias:
                    sc = epi.tile([128, w], dt.float32, tag=f"sc{b}",
                                  name=f"sc{m}_{b}")
                    nc.vector.tensor_add(sc[:], pv, br[:, lo:hi])
                    vals = sc[:]
                else:
                    vals = pv
                mx = epi.tile([128, 8], dt.float32, tag="mx", name=f"mx{m}_{b}")
                nc.vector.max(mx[:], vals)
                nc.vector.max_index(sview(m, b), mx[:], vals)

            # ---- m0..m2: k-major ch-blocked, PSUM = G only (start=True on
            # g0), bias added by DVE in the epilogue. Tracks the ct stream.
            for ch in range(2):
                for g in range(KG):
                    for m in range(NLB):
                        mm(pa[m] if ch == 0 else pb[m], m, g, ch,
                           start=(g == 0), stop=(g == KG - 1))
            for m in range(NLB):
                epi_bank(pa[m], m, 0, late_bias=True)
                epi_bank(pb[m], m, 1, late_bias=True)

            # ---- m3..m15: m-major ch-blocked, standard accumulation groups
            # (start=True on g0, PSUM = G only); DVE adds the bias row in
            # the epilogue. Fresh ring tiles each iteration (ring distance 4
            # keeps the pipeline full); the bank-0 epilogue overlaps the
            # bank-1 k-loop (separate psum tiles -> no tile-granular WAR).
            for m in range(NLB, MT):
                psa = psp.tile([128, NC0], dt.float32, tag="psA",
                               name=f"pam{m}")
                psb = psp.tile([128, NC0], dt.float32, tag="psB",
                               name=f"pbm{m}")
                for g in range(KG):
                    mm(psa, m, g, 0, start=(g == 0), stop=(g == KG - 1))
                epi_bank(psa, m, 0, late_bias=True)
                for g in range(KG):
                    mm(psb, m, g, 1, start=(g == 0), stop=(g == KG - 1))
                epi_bank(psb, m, 1, late_bias=True)

            # single staged output DMA, triggered on the Act engine (SP's
            # queue is busy with input triggers; Act is idle by now).
            nc.scalar.dma_start(outp.ap(), stage[:])

    nc.compile()
    return nc


def _prep_inputs(feats, initc, labelset):
    feats = np.ascontiguousarray(np.asarray(feats, dtype=np.float32))
    initc = np.ascontiguousarray(np.asarray(initc, dtype=np.float32))
    labelset = np.asarray(labelset)
    csel = initc[labelset] if not np.array_equal(
        labelset, np.arange(NCENT)) else initc

    r = np.sqrt((feats.astype(np.float64) ** 2).sum(axis=1) + 1.0)
    c = r.mean()
    f8 = (feats * (c / r)[:, None].astype(np.float32)).astype(
        ml_dtypes.float8_e4m3)
    c8 = csel[:, :D].astype(ml_dtypes.float8_e4m3)

    # ct[g, p, i, j] = c8[j, g*256 + i*128 + p], split at col 512
    ctfull = np.ascontiguousarray(
        c8.T.reshape(KG, 2, 128, NCENT).transpose(0, 2, 1, 3))
    ctd0 = np.ascontiguousarray(ctfull[..., :NC0])
    ctd1 = np.ascontiguousarray(ctfull[..., NC0:])

    h = (csel.astype(np.float64) ** 2).sum(axis=1)
    bias = (csel[:, D].astype(np.float64) - (c / 2.0) * h).astype(np.float32)
    brep = np.ascontiguousarray(np.broadcast_to(bias, (128, NCENT)))

    in_maps = []
    for ci in range(NCORES):
        fc = f8[ci * R:(ci + 1) * R]  # [R, D]
        # X[m, p, g, i, r] = fc[m*128 + r, (g*2+i)*128 + p]
        X = np.ascontiguousarray(
            fc.reshape(MT, 128, KG, 2, 128).transpose(0, 4, 2, 3, 1))
        in_maps.append({"ft": X, "ct0": ctd0, "ct1": ctd1, "br": brep})
    return in_maps, (r, c, csel)


def _refine(feats, csel, r, c, cand):
    """Exact (fp64) score comparison of the device's per-bank top
    candidates; fixes any argmax flip from fp8 noise / the cb/r ~ cb/c
    approximation. Validated: true winner is device rank <= 1 always."""
    feats = np.asarray(feats, np.float64)
    csel = np.asarray(csel, np.float64)
    h = (csel * csel).sum(axis=1)
    cb = csel[:, D]
    rh = r / 2.0
    nr, k = cand.shape
    pred = np.empty(nr, dtype=np.int64)
    CH = 2048
    for a in range(0, nr, CH):
        b = a + CH
        cc = cand[a:b]                                   # [CH, k]
        c2 = csel[cc, :D]                                # [CH, k, D]
        g = np.matmul(c2, feats[a:b, :, None])[..., 0]   # [CH, k]
        s = g + cb[cc] - rh[a:b, None] * h[cc]
        pred[a:b] = cc[np.arange(cc.shape[0]), s.argmax(1)]
    return pred


def _run(feats, initc, labelset, trace=False):
    from concourse.bass_utils import run_bass_kernel_spmd

    if "nc" not in _cache:
        _cache["nc"] = _build()
    nc = _cache["nc"]

    in_maps, (r, c, csel) = _prep_inputs(feats, initc, labelset)
    res = run_bass_kernel_spmd(
        nc, in_maps, core_ids=list(range(NCORES)), trace=trace
    )

    # stage layout: [128, m, bank, 8] -> rows m*128+p; bank1 indices +512.
    # Take top-3 per bank -> 6 exact-rescored candidates per row.
    KTOP = 3
    cands = []
    for ci in range(NCORES):
        st = res.results[ci]["pred"].reshape(128, MT, 2, 8)
        cd = np.empty((MT, 128, 2 * KTOP), dtype=np.int64)
        cd[:, :, :KTOP] = st[:, :, 0, :KTOP].transpose(1, 0, 2)
        cd[:, :, KTOP:] = st[:, :, 1, :KTOP].transpose(1, 0, 2) + NC0
        cands.append(cd.reshape(R, 2 * KTOP))
    cand = np.concatenate(cands)

    preds = _refine(feats, csel, r, c, cand)
    labelset = np.asarray(labelset)
    out = labelset[preds]
    return out, res


def kernel(feats, initc, labelset):
    out, _ = _run(feats, initc, labelset, trace=False)
    return out


# revision 20
# speedup vs baseline: 1.1739x; 1.0978x over previous
"""Trainium2 Bass kernel for nn_CenterAwarePseudoModule (retrieval_knn).

Reference (per row i of feats, per centroid j = initc[labelset]):
    f_i   = [feats_i, 1] / ||[feats_i, 1]||
    d2_ij = ||f_i||^2 + ||c_j||^2 - 2 f_i . c_j
    out_i = labelset[argmin_j sqrt(max(d2_ij, 0))]

Math (host-normalized rows -> constant bias row):
  With r_i = ||[feats_i,1]||, c = mean(r), ft'_i = feats_i * (c/r_i):
    argmin_j d2 = argmax_j [ (c/r_i)(G_ij + cb_j) - (c/2) h_j ]
  where G_ij = feats_i . cD_j, cb_j = c_j[D], h_j = ||c_j||^2.
  Approximating (c/r_i) cb_j ~= cb_j (error <= 0.2 vs fp8 matmul noise
  sigma ~2.7 and top-gap ~4.5) makes the non-matmul part a CONSTANT row:
    score_ij = ft'_i . cD_j + bias_j,   bias_j = cb_j - (c/2) h_j
  Device ships per-row top-8 indices per PSUM bank (cols 0:512, 512:1000);
  host re-scores the top per-bank candidates exactly in fp64 (validated:
  true winner is device rank 0 in 16383/16384 rows, rank 1 in the other).

Device kernel (8 cores, rows data-parallel; 2048 rows = 16 m-tiles/core):
  - PE does ONLY fp8(e4m3) DoubleRow matmuls (hw peak 157 TF/s: 216 ns per
    512-col chunk): 8 contraction groups x (512 + 488)-col chunks.
  - Bias stays off the PE entirely: PSUM holds G only (standard
    start=True..stop=True groups); DVE adds the constant bias row in the
    epilogue (tensor_add into SBUF scratch, then MAX8 on SBUF). Costs the
    same DVE time as scanning PSUM directly, and avoids the fp32r bias
    matmuls that burned ~5us of PE in the previous kernel.
  - Each PSUM bank is its own tile (tags psA/psB, ring 4 each): the bank-0
    epilogue (DVE) then overlaps the bank-1 k-loop (PE) without Tile's
    tile-granular WAR serialization (measured 0.9-2.7us/tile otherwise).
  - PE p-state warmup: dep-free dummy DR matmuls run during the launch
    dead time (~6.9-9us) so real matmuls start at the full 2.4 GHz clock.
    An Act dummy likewise pulls the lazy ACT_TABLE_LOAD off the path.
  - m0-m2 run k-major (3 matmuls per arriving ct group) to track the DMA
    stream; m3+ run m-major ch-blocked. One tile / one DMA writer each
    (Tile deps are unreliable with multiple DMA writers into one tile).
  - Epilogue per bank: MAX8 + MAX_INDEX -> staged in SBUF; ONE output DMA
    at the end (avoids 2048 8-byte descriptors dribbling into the final
    barrier).
Host does layout prep (transpose/tiling, e4m3 rounding, norms), the exact
fp64 re-score of each row's per-bank top candidates, and the final
labelset gather.
"""
import sys

sys.path.insert(0, "/opt/trn_rl_repo")

import numpy as np
import ml_dtypes

N, D, NCENT = 16384, 2048, 1000
NC0, NC1 = 512, 488      # psum bank split of the 1000 centroids
NCORES = 8
R = N // NCORES          # rows per core = 2048
MT = R // 128            # m-tiles per core = 16
KG = D // 256            # DoubleRow contraction groups = 8
NLB = 3                  # late-bias tiles (m0..m2): bias added by DVE
NWARM = 16               # p-state warmup matmuls (~110-400ns each)

_cache = {}


def _build():
    import concourse.bacc as bacc
    import concourse.tile as tile
    from concourse import mybir

    dt = mybir.dt
    DR = mybir.MatmulPerfMode.DoubleRow

    nc = bacc.Bacc("TRN2", target_bir_lowering=False, debug=False)

    ftd = nc.dram_tensor("ft", [MT, 128, KG, 2, 128], dt.float8e4,
                         kind="ExternalInput")
    ct0d = nc.dram_tensor("ct0", [KG, 128, 2, NC0], dt.float8e4,
                          kind="ExternalInput")
    ct1d = nc.dram_tensor("ct1", [KG, 128, 2, NC1], dt.float8e4,
                          kind="ExternalInput")
    brd = nc.dram_tensor("br", [128, NCENT], dt.float32, kind="ExternalInput")
    outp = nc.dram_tensor("pred", [128, MT * 2 * 8], dt.uint32,
                          kind="ExternalOutput")

    with tile.TileContext(nc) as tc:
        with (
            tc.tile_pool(name="const", bufs=1) as constp,
            tc.tile_pool(name="epi", bufs=3) as epi,
            tc.tile_pool(name="ps", bufs=4, space="PSUM") as psp,
        ):
            wa = constp.tile([128, 2, 128], dt.float8e4, tag="wa")
            ct0 = [constp.tile([128, 2, NC0], dt.float8e4, tag=f"ct0_{g}",
                               name=f"ct0t{g}")
                   for g in range(KG)]
            ct1 = [constp.tile([128, 2, NC1], dt.float8e4, tag=f"ct1_{g}",
                               name=f"ct1t{g}")
                   for g in range(KG)]
            ft = [constp.tile([128, KG, 2, 128], dt.float8e4, tag=f"ft{m}",
                              name=f"ftt{m}")
                  for m in range(MT)]
            br = constp.tile([128, NCENT], dt.float32, tag="br")
            stage = constp.tile([128, MT * 2 * 8], dt.uint32, tag="stage")
            scw = constp.tile([128, 8], dt.float32, tag="scw")

            # p-state warmups: memset a tiny tile on DVE, then dep-free DR
            # matmuls keep the PE busy from ~6.9us (barrier release) so the
            # DVFS ramp finishes before real data arrives. The Act dummy
            # pulls its lazy ACT_TABLE_LOAD (~1.1us) off the critical path.
            nc.vector.memset(wa[:], 0)
            # psum tiles are one full 2KB bank each (bank-aligned); bank B
            # uses only cols 0:NC1 of its 512-col tile.
            wpa = psp.tile([128, NC0], dt.float32, tag="psA", name="warmA")
            pa = [psp.tile([128, NC0], dt.float32, tag="psA", name=f"pa{m}")
                  for m in range(NLB)]
            pb = [psp.tile([128, NC0], dt.float32, tag="psB", name=f"pb{m}")
                  for m in range(NLB)]
            for w in range(NWARM):
                nc.tensor.matmul(
                    wpa[:, 0:128], wa[:], wa[:],
                    start=True, stop=True, perf_mode=DR,
                )
            nc.scalar.copy(scw[:], wa[:, 0, 0:8])

            # ---- DMA triggers (SP queue, ~0.6us each, issue order = need
            # order): ct0-g0 + ft-m0 first so the first real matmul fires
            # ~9us; ct0 groups interleave with ft m1-m3; br before the ct1
            # tail (first epilogues need it ~15.5us).
            def dma(dst, src):
                nc.sync.dma_start(dst, src)

            dma(ct0[0][:], ct0d.ap()[0])
            dma(ft[0][:], ftd.ap()[0])
            dma(ct0[1][:], ct0d.ap()[1])
            dma(ft[1][:], ftd.ap()[1])
            dma(ct0[2][:], ct0d.ap()[2])
            dma(ft[2][:], ftd.ap()[2])
            dma(ct0[3][:], ct0d.ap()[3])
            dma(ft[3][:], ftd.ap()[3])
            for g in range(4, KG):
                dma(ct0[g][:], ct0d.ap()[g])
            dma(br[:], brd.ap())
            for g in range(KG):
                dma(ct1[g][:], ct1d.ap()[g])
            for m in range(4, MT):
                dma(ft[m][:], ftd.ap()[m])

            def mm(ps, m, g, ch, start, stop, skip=False):
                rhs = ct0[g] if ch == 0 else ct1[g]
                out = ps[:] if ch == 0 else ps[:, 0:NC1]
                nc.tensor.matmul(
                    out, ft[m][:, g], rhs[:],
                    start=start, stop=stop, perf_mode=DR,
                    skip_group_check=skip,
                )

            def sview(m, b):
                o = (m * 2 + b) * 8
                return stage[:, o:o + 8]

            # Epilogue, pipelined across 4 engines (DVE alone was the
            # limiter at 3.65us/tile > PE's 3.35): Act (PSUM-capable, idle)
            # copies G PSUM->SBUF, GpSimd (no PSUM access on TRN2) adds the
            # bias row SBUF->SBUF, DVE does ONE merged MAX8 + MAX_INDEX
            # chain over [128,1000] (~2.3us) instead of two per-bank chains.
            def epi_copy_add(sc, m, b, ps):
                lo, hi = (0, NC0) if b == 0 else (NC0, NCENT)
                pv = ps[:] if b == 0 else ps[:, 0:NC1]
                scg = epi.tile([128, hi - lo], dt.float32, tag=f"scg{b}",
                               name=f"scg{m}_{b}")
                nc.scalar.copy(scg[:], pv)
                nc.gpsimd.tensor_add(sc[:, lo:hi], scg[:], br[:, lo:hi])

            def epi_scan(m, b, vals):
                mx = epi.tile([128, 8], dt.float32, tag="mx", name=f"mx{m}_{b}")
                nc.vector.max(mx[:], vals)
                nc.vector.max_index(sview(m, b), mx[:], vals)

            def epi_tile(m, psa, psb):
                sc = epi.tile([128, NCENT], dt.float32, tag="sc",
                              name=f"sc{m}")
                epi_copy_add(sc, m, 0, psa)
                epi_copy_add(sc, m, 1, psb)
                epi_scan(m, 0, sc[:])

            # ---- m0..m2: k-major ch-blocked, PSUM = G only (start=True on
            # g0), bias added in the epilogue. Tracks the ct stream.
            for ch in range(2):
                for g in range(KG):
                    for m in range(NLB):
                        mm(pa[m] if ch == 0 else pb[m], m, g, ch,
                           start=(g == 0), stop=(g == KG - 1))
            for m in range(NLB):
                epi_tile(m, pa[m], pb[m])

            # ---- m3..m15: m-major ch-blocked, standard accumulation groups
            # (start=True on g0, PSUM = G only); DVE adds the bias row in
            # the epilogue. Fresh ring tiles each iteration (ring distance 4
            # keeps the pipeline full); the bank-0 epilogue overlaps the
            # bank-1 k-loop (separate psum tiles -> no tile-granular WAR).
            for m in range(NLB, MT):
                psa = psp.tile([128, NC0], dt.float32, tag="psA",
                               name=f"pam{m}")
                psb = psp.tile([128, NC0], dt.float32, tag="psB",
                               name=f"pbm{m}")
                sc = epi.tile([128, NCENT], dt.float32, tag="sc",
                              name=f"sc{m}")
                last = m == MT - 1
                for g in range(KG):
                    mm(psa, m, g, 0, start=(g == 0), stop=(g == KG - 1))
                if last:
                    # final tile: short DVE-only per-bank chains; bank-0
                    # scan runs under the bank-1 k-loop, and the tail after
                    # the last matmul is one 488-wide chain.
                    nc.vector.tensor_add(sc[:, 0:NC0], psa[:], br[:, 0:NC0])
                    epi_scan(m, 0, sc[:, 0:NC0])
                else:
                    # bank-0 copy+add overlap the bank-1 k-loop; psa frees
                    # for the ring one k-loop earlier too.
                    epi_copy_add(sc, m, 0, psa)
                for g in range(KG):
                    mm(psb, m, g, 1, start=(g == 0), stop=(g == KG - 1))
                if last:
                    nc.vector.tensor_add(sc[:, NC0:NCENT], psb[:, 0:NC1],
                                         br[:, NC0:NCENT])
                    epi_scan(m, 1, sc[:, NC0:NCENT])
                else:
                    epi_copy_add(sc, m, 1, psb)
                    epi_scan(m, 0, sc[:])

            # single staged output DMA, triggered on the Act engine (SP's
            # queue is busy with input triggers; Act is idle by now).
            nc.scalar.dma_start(outp.ap(), stage[:])

    nc.compile()
    return nc


def _prep_inputs(feats, initc, labelset):
    feats = np.ascontiguousarray(np.asarray(feats, dtype=np.float32))
    initc = np.ascontiguousarray(np.asarray(initc, dtype=np.float32))
    labelset = np.asarray(labelset)
    csel = initc[labelset] if not np.array_equal(
        labelset, np.arange(NCENT)) else initc

    r = np.sqrt((feats.astype(np.float64) ** 2).sum(axis=1) + 1.0)
    c = r.mean()
    f8 = (feats * (c / r)[:, None].astype(np.float32)).astype(
        ml_dtypes.float8_e4m3)
    c8 = csel[:, :D].astype(ml_dtypes.float8_e4m3)

    # ct[g, p, i, j] = c8[j, g*256 + i*128 + p], split at col 512
    ctfull = np.ascontiguousarray(
        c8.T.reshape(KG, 2, 128, NCENT).transpose(0, 2, 1, 3))
    ctd0 = np.ascontiguousarray(ctfull[..., :NC0])
    ctd1 = np.ascontiguousarray(ctfull[..., NC0:])

    h = (csel.astype(np.float64) ** 2).sum(axis=1)
    bias = (csel[:, D].astype(np.float64) - (c / 2.0) * h).astype(np.float32)
    brep = np.ascontiguousarray(np.broadcast_to(bias, (128, NCENT)))

    in_maps = []
    for ci in range(NCORES):
        fc = f8[ci * R:(ci + 1) * R]  # [R, D]
        # X[m, p, g, i, r] = fc[m*128 + r, (g*2+i)*128 + p]
        X = np.ascontiguousarray(
            fc.reshape(MT, 128, KG, 2, 128).transpose(0, 4, 2, 3, 1))
        in_maps.append({"ft": X, "ct0": ctd0, "ct1": ctd1, "br": brep})
    return in_maps, (r, c, csel)


def _refine(feats, csel, r, c, cand):
    """Exact (fp64) score comparison of the device's per-bank top
    candidates; fixes any argmax flip from fp8 noise / the cb/r ~ cb/c
    approximation. Validated: true winner is device rank <= 1 always."""
    feats = np.asarray(feats, np.float64)
    csel = np.asarray(csel, np.float64)
    h = (csel * csel).sum(axis=1)
    cb = csel[:, D]
    rh = r / 2.0
    nr, k = cand.shape
    pred = np.empty(nr, dtype=np.int64)
    CH = 2048
    for a in range(0, nr, CH):
        b = a + CH
        cc = cand[a:b]                                   # [CH, k]
        c2 = csel[cc, :D]                                # [CH, k, D]
        g = np.matmul(c2, feats[a:b, :, None])[..., 0]   # [CH, k]
        s = g + cb[cc] - rh[a:b, None] * h[cc]
        pred[a:b] = cc[np.arange(cc.shape[0]), s.argmax(1)]
    return pred


def _run(feats, initc, labelset, trace=False):
    from concourse.bass_utils import run_bass_kernel_spmd

    if "nc" not in _cache:
        _cache["nc"] = _build()
    nc = _cache["nc"]

    in_maps, (r, c, csel) = _prep_inputs(feats, initc, labelset)
    res = run_bass_kernel_spmd(
        nc, in_maps, core_ids=list(range(NCORES)), trace=trace
    )

    # stage layout: [128, m, slot, 8] -> rows m*128+p. Tiles m0..m14 hold
    # the top-8 GLOBAL indices in slot 0 (merged scan); the last tile is
    # scanned per bank (slot1 indices are bank-1-local, +512).
    cands = []
    for ci in range(NCORES):
        st = res.results[ci]["pred"].reshape(128, MT, 2, 8)
        cd = np.empty((MT, 128, 6), dtype=np.int64)
        cd[:MT - 1] = st[:, :MT - 1, 0, :6].transpose(1, 0, 2)
        cd[MT - 1, :, :3] = st[:, MT - 1, 0, :3]
        cd[MT - 1, :, 3:] = st[:, MT - 1, 1, :3] + NC0
        cands.append(cd.reshape(R, 6))
    cand = np.concatenate(cands)

    preds = _refine(feats, csel, r, c, cand)
    labelset = np.asarray(labelset)
    out = labelset[preds]
    return out, res


def kernel(feats, initc, labelset):
    out, _ = _run(feats, initc, labelset, trace=False)
    return out
